# revision 14
# baseline (speedup 1.0000x reference)
"""Trainium2 Bass kernel for a 2-layer dense GCN block:

    z = x.reshape(B, N, F)                     # B=4, N=8192, F=64
    for i in range(2):
        z = relu((A @ z) @ W_i)                # A: [N, N] dense
    return z

Strategy (8 NeuronCores, SPMD):
  * Shard the output rows (m) of A @ Z across cores: core j owns rows
    [1024*j, 1024*(j+1)) and keeps its A^T column-slice (bf16, 16 MiB)
    resident in SBUF for BOTH layers, so A is read from HBM exactly once.
  * HOST-SIDE REPACK: A^T / Z0 / the output use per-partition contiguous
    tiled layouts (8 KiB DMA lines).  W1 is FOLDED INTO Z0 on the host
    ((A z) W1 = A (z W1)); layer 1's tail applies relu then the
    block-diag W2 ((A z1) W2 = A (z1 W2)), and layer 2's tail is a bare
    relu with the output left in [c, m] layout the host untangles.
  * ROTATED PER-CORE CHUNK LAYOUT: core c's A^T / Z0 contraction chunks
    are host-reordered to [(c+1+j) % 8 for j in 0..6] + [c], putting the
    core's OWN n-rows at static slot 7.  Layer 1 is order-invariant; for
    layer 2 this makes the 8 "self" n-tiles addressable from the LOCAL
    z1' send tiles with no gather dependency.  They are issued first, so
    the PE crunches ~9us of real work in the window where it previously
    idled waiting for the first gather (whose ~18us stall re-throttled
    the HAM clock gate to 1.2 GHz for the start of layer 2).
  * The gathered peer blocks are restaged with DYNAMIC row offsets
    (partition_id + ds): slot s <- gathered block (rank+1+s) % 8, so the
    slot order matches the rotated A^T layout on every core.
  * Z is [n, c] with c = b*F + f (256 columns).  Layer matmuls compute
    H^T[c, m] = sum_n Z[n, c] * A^T[n, m] (lhsT = Z tile, rhs = A^T
    half tile [128, 512], fp32 PSUM accum).
  * Layer 1 runs as TWO m-half passes in DMA arrival order, one 1 MiB
    AllGather per half (CC ops have a ~13us serial floor; NG=4 measured
    worse).  mh=1 stores ride the gpsimd queue (its DMA-completion
    semaphore pool is untouched by load/restage traffic).
  * A 1-element relu at kernel start absorbs the ~2.7us ACT_TABLE_LOAD.
  * bf16 operands / fp32 accumulation (measured ~0.5% rel-l2 vs the
    fp32 reference).  Final output is fp32 (reassembled on the host).
"""

import contextlib

import numpy as np
import ml_dtypes

import concourse.mybir as mybir
import concourse.tile as tile
from concourse import bacc
from concourse.bass import ds
from concourse.bass_utils import run_bass_kernel_spmd

BF16 = ml_dtypes.bfloat16

NCORES = 8
B, N, F, L = 4, 8192, 64, 2
C = B * F                      # 256 columns of the Z matrix
M_CORE = N // NCORES           # 1024 output rows per core
NT = N // 128                  # 64 contraction tiles of 128
MT = M_CORE // 128             # 8 output-row tiles of 128 per core
NG = 2                         # one AllGather per m-half
MPG = MT // NG                 # m-tiles per gather slice (4)
TPC = 8                        # n-tiles per DMA chunk
KCH = NT // TPC                # 8 chunks
NSLOT = NCORES - 1             # restaged peer slots (self excluded)
STAG2 = 16                     # layer-2 tail sweep tiles

_CACHED = {}


def _build_program():
    nc = bacc.Bacc("TRN2", target_bir_lowering=False, debug=False,
                   num_devices=NCORES)
    dt = mybir.dt

    # host-repacked inputs: per-partition-contiguous tiled layouts
    atr_d = nc.dram_tensor("atr", [2 * KCH, 128, TPC * 512], dt.bfloat16,
                           kind="ExternalInput")
    z0r_d = nc.dram_tensor("z0r", [KCH, 128, TPC * C], dt.bfloat16,
                           kind="ExternalInput")
    w_d = nc.dram_tensor("w", [128, 128], dt.bfloat16, kind="ExternalInput")
    # output: z2^T slabs [c-in-ch, (mh, ch) x 512 m]; host reassembles
    out_d = nc.dram_tensor("out", [128, 4 * 512], dt.bfloat16,
                           kind="ExternalOutput")

    z1_loc = [nc.dram_tensor(f"z1_loc{g}", [128, MPG * C], dt.bfloat16)
              for g in range(NG)]
    z1g = [nc.dram_tensor(f"z1g{g}", [NCORES * 128, MPG * C], dt.bfloat16,
                          addr_space="Shared")
           for g in range(NG)]

    with tile.TileContext(nc) as tc:
        with tc.tile_pool(name="a_res", bufs=1) as a_pool, \
             tc.tile_pool(name="z_res", bufs=1) as z_pool, \
             tc.tile_pool(name="z1_res", bufs=1) as z1_pool, \
             tc.tile_pool(name="snd", bufs=1) as snd_pool, \
             tc.tile_pool(name="wk", bufs=1) as w_pool, \
             tc.tile_pool(name="ht", bufs=3, space="PSUM") as psh_pool, \
             tc.tile_pool(name="pz", bufs=2, space="PSUM") as psz_pool, \
             tc.tile_pool(name="hsb", bufs=2) as hsb_pool:

            # per-engine core rank + rotated gather-block indices for the
            # dynamic restages: slot s holds peer (rank+1+s) % 8
            p_sync = nc.sync.partition_id()
            p_gps = nc.gpsimd.partition_id()
            p_scal = nc.scalar.partition_id()
            bs_sync = [nc.sync.snap((p_sync + 1 + s) % NCORES,
                                    min_val=0, max_val=NCORES - 1)
                       for s in range(NSLOT)]
            bs_scal = [nc.scalar.snap((p_scal + 1 + s) % NCORES,
                                      min_val=0, max_val=NCORES - 1)
                       for s in range(NSLOT)]
            b0_gps = nc.gpsimd.snap((p_gps + 1) % NCORES,
                                    min_val=0, max_val=NCORES - 1)

            w_sb = w_pool.tile([128, 128], dt.bfloat16, tag="w")
            nc.scalar.dma_start(out=w_sb[:], in_=w_d[:])
            # ACT_TABLE_LOAD warmup: first relu pays ~2.7us table load
            wrm_sb = w_pool.tile([128, 1], dt.bfloat16, tag="wrm")
            nc.scalar.activation(wrm_sb[:], w_sb[:, :1],
                                 mybir.ActivationFunctionType.Relu)

            ath_sb = {(mh, j): a_pool.tile([128, TPC * 512], dt.bfloat16,
                                           tag=f"ath{mh}{j}",
                                           name=f"ath_sb{mh}{j}")
                      for mh in range(2) for j in range(KCH)}
            z_sb = [z_pool.tile([128, TPC * C], dt.bfloat16,
                                tag=f"z{j}", name=f"z_sb{j}")
                    for j in range(KCH)]
            z1_sb = [z1_pool.tile([128, NSLOT * MPG * C], dt.bfloat16,
                                  tag=f"z1s{g}", name=f"z1_sb{g}")
                     for g in range(NG)]
            # contiguous per-half send tiles (tails write slices of these;
            # layer 2's self n-tiles also read them directly)
            z1snd = [snd_pool.tile([128, MPG * C], dt.bfloat16,
                                   tag=f"z1snd{mh}", name=f"z1snd{mh}")
                     for mh in range(2)]
            z2snd = snd_pool.tile([128, 4 * 512], dt.bfloat16,
                                  tag="z2snd", name="z2snd")

            # pass-0 feed: Z0 rides the scalar queue (done by ~25us, well
            # before the tail's store DMAs need it) while A-mh0 rides
            # sync -- two DGE queues push descriptors through the slow
            # first-5us ramp instead of one.  First chunk split so the
            # first matmul starts ~6us earlier.
            nc.scalar.dma_start(out=z_sb[0][:, :C], in_=z0r_d[0][:, :C])
            nc.sync.dma_start(out=ath_sb[0, 0][:, :512],
                              in_=atr_d[0][:, :512])
            nc.scalar.dma_start(out=z_sb[0][:, C:2 * C],
                                in_=z0r_d[0][:, C:2 * C])
            nc.sync.dma_start(out=ath_sb[0, 0][:, 512:2 * 512],
                              in_=atr_d[0][:, 512:2 * 512])
            nc.scalar.dma_start(out=z_sb[0][:, 2 * C:],
                                in_=z0r_d[0][:, 2 * C:])
            nc.sync.dma_start(out=ath_sb[0, 0][:, 2 * 512:],
                              in_=atr_d[0][:, 2 * 512:])
            for j in range(1, KCH):
                nc.scalar.dma_start(out=z_sb[j][:], in_=z0r_d[j])
                nc.sync.dma_start(out=ath_sb[0, j][:], in_=atr_d[j])

            def z0_tile(t, ch):
                """lhsT: Z0[n-slot-tile t, c-half ch] -> [128, 128] bf16."""
                j, tt = divmod(t, TPC)
                return z_sb[j][:, tt * C + ch * 128: tt * C + ch * 128 + 128]

            def z1_tile(t, ch):
                """lhsT: restaged peer Z1 for slot-tile t (slot 0..6)."""
                s, r = divmod(t, MT)
                g, i = divmod(r, MPG)
                base = s * (MPG * C) + i * C + ch * 128
                return z1_sb[g][:, base: base + 128]

            def z1self_tile(r, ch):
                """lhsT: the core's own Z1 (slot 7) straight from the
                send tiles -- no gather dependency."""
                mh, io = divmod(r, MPG)
                return z1snd[mh][:, io * C + ch * 128: io * C + ch * 128 + 128]

            def at_tile(t, mh):
                """rhs: A^T[n-slot-tile t, m-half mh] -> [128, 512] bf16."""
                j, tt = divmod(t, TPC)
                return ath_sb[mh, j][:, tt * 512:(tt + 1) * 512]

            def l1_tail(mh, h_ps, hook):
                """relu(h1) -> bf16, apply block-diag W2 (which also
                transposes [c,m] -> [m,c]), round into the half's send
                tile, then the caller's per-m-tile hook."""
                with tc.high_priority():
                    hr = [hsb_pool.tile([128, 512], dt.bfloat16,
                                        tag=f"h{ch}", name=f"hr_sb_{ch}{mh}")
                          for ch in range(2)]
                    for ch in range(2):
                        nc.scalar.activation(
                            hr[ch][:], h_ps[ch, mh][:],
                            mybir.ActivationFunctionType.Relu)
                    for io in range(MPG):
                        sl = slice(io * 128, (io + 1) * 128)
                        z_ps = psz_pool.tile([128, C], dt.float32, tag="zps",
                                             name=f"z_ps_{mh}{io}")
                        for ch in range(2):
                            nc.tensor.matmul(
                                z_ps[:, ch * 128:(ch + 1) * 128],
                                hr[ch][:, sl], w_sb[:],
                                start=True, stop=True)
                        nc.vector.tensor_copy(
                            z1snd[mh][:, io * C:(io + 1) * C], z_ps[:])
                        hook(mh, io, io + 1)

            # ---- layer 1: two m-half passes in DMA arrival order ----
            def l1_hook(mh, lo, hi):
                # 2KiB-line store of part of the half's z1'; after the
                # last part, the gather and the rotated per-slot restage
                # into SBUF.  mh=1's stores go on the gpsimd queue: its
                # DMA-completion semaphore pool is untouched by the
                # load/restage traffic, so the stores cannot inherit a
                # ring-wait on a gather-gated DMA.
                st_eng = nc.sync if mh == 0 else nc.gpsimd
                st_eng.dma_start(out=z1_loc[mh][:, lo * C:hi * C],
                                 in_=z1snd[mh][:, lo * C:hi * C])
                if hi < MPG:
                    return
                nc.gpsimd.collective_compute(
                    "AllGather",
                    mybir.AluOpType.bypass,
                    replica_groups=[list(range(NCORES))],
                    ins=[z1_loc[mh].ap().opt()],
                    outs=[z1g[mh].ap().opt()],
                )
                # Slot 0 (peer rank+1) in m-tile pieces so layer 2's
                # first post-self matmul sees minimum staging latency.
                # g0's pieces are deferred to after gather-1's doorbell,
                # behind a scheduler fence, and ride the gpsimd queue.
                if mh == 1:
                    tc.no_sync_barrier()
                    for i in range(MPG):
                        nc.gpsimd.dma_start(
                            out=z1_sb[0][:, i * C:(i + 1) * C],
                            in_=z1g[0][ds(b0_gps * 128, 128)][:,
                                                              i * C:
                                                              (i + 1) * C])
                    for half in range(2):
                        sl = slice(half * MPG * C // 2,
                                   (half + 1) * MPG * C // 2)
                        nc.sync.dma_start(
                            out=z1_sb[1][:, sl],
                            in_=z1g[1][ds(bs_sync[0] * 128, 128)][:, sl])
                # remaining slots alternate sync/scalar so the post-gather
                # restage burst drains two DGE queues in parallel
                for s in range(1, NSLOT):
                    eng, bsv = ((nc.sync, bs_sync[s]) if s % 2
                                else (nc.scalar, bs_scal[s]))
                    eng.dma_start(
                        out=z1_sb[mh][:, s * MPG * C:(s + 1) * MPG * C],
                        in_=z1g[mh][ds(bsv * 128, 128)])

            h_ps1 = {(ch, mh): psh_pool.tile([128, 512], dt.float32,
                                             tag=f"hps{ch}",
                                             name=f"hps_1_{ch}{mh}")
                     for ch in range(2) for mh in range(2)}
            for mh in range(2):
                for t in range(NT):
                    for ch in range(2):
                        nc.tensor.matmul(
                            h_ps1[ch, mh][:], z0_tile(t, ch), at_tile(t, mh),
                            start=(t == 0), stop=(t == NT - 1))
                if mh == 0:
                    # first two A-mh1 chunks ahead of the mh0 z1 stores
                    # (pass 1 needs them before the stores' data exists)
                    for j in range(2):
                        nc.sync.dma_start(out=ath_sb[1, j][:],
                                          in_=atr_d[KCH + j])
                l1_tail(mh, h_ps1, l1_hook)
                if mh == 0:
                    # rest of the A-mh1 stream AFTER the mh0 z1 stores so
                    # the stores' packets do not trail the 8MiB load
                    for j in range(2, KCH):
                        nc.sync.dma_start(out=ath_sb[1, j][:],
                                          in_=atr_d[KCH + j])

            # ---- layer 2: self-first, then gather arrival order ----
            h_ps2 = {(ch, mh): psh_pool.tile([128, 512], dt.float32,
                                             tag=f"hps{ch}",
                                             name=f"hps_2_{ch}{mh}")
                     for ch in range(2) for mh in range(2)}
            # Scheduler fence: orders every engine's queue here so layer
            # 2's matmuls cannot head-of-line block layer-1 mh1's tail /
            # the second gather's doorbell.
            tc.no_sync_barrier()
            # self n-tiles (slot 7): local z1' only -- these fill the PE
            # gap while the first gather is still in flight, keeping the
            # HAM clock gate warm
            for r in range(MT):
                for ch in range(2):
                    for mh in range(2):
                        nc.tensor.matmul(
                            h_ps2[ch, mh][:], z1self_tile(r, ch),
                            at_tile(NSLOT * TPC + r, mh),
                            start=(r == 0), stop=False)
            t2 = [s * MT + g * MPG + i
                  for g in range(NG) for s in range(NSLOT) for i in range(MPG)]
            head, sweep = t2[:len(t2) - STAG2], t2[len(t2) - STAG2:]
            for t in head:
                for ch in range(2):
                    for mh in range(2):
                        nc.tensor.matmul(
                            h_ps2[ch, mh][:], z1_tile(t, ch), at_tile(t, mh),
                            start=False, stop=False)
            for mh in range(2):
                for si, t in enumerate(sweep):
                    for ch in range(2):
                        nc.tensor.matmul(
                            h_ps2[ch, mh][:], z1_tile(t, ch), at_tile(t, mh),
                            start=False, stop=(si == STAG2 - 1))
                prio = (contextlib.nullcontext() if mh == 1
                        else tc.high_priority())
                with prio:
                    for ch in range(2):
                        off = (mh * 2 + ch) * 512
                        nc.scalar.activation(
                            z2snd[:, off:off + 512],
                            h_ps2[ch, mh][:],
                            mybir.ActivationFunctionType.Relu)
                        nc.scalar.dma_start(
                            out=out_d[:, off:off + 512],
                            in_=z2snd[:, off:off + 512])

    nc.compile()
    return nc


def _prep_inputs(x, net_params, A):
    a_bf = A.astype(BF16)
    w = net_params.astype(np.float32).reshape(L, F, F)
    z0 = np.ascontiguousarray(x.transpose(1, 0, 2))          # [N, B, F] f32
    # fold W1 into z0:  (A z0) W1 == A (z0 W1)
    z0p = np.einsum('nbf,fg->nbg', z0, w[0]).reshape(N, C).astype(BF16)
    # z0 repack: [chunk j][partition p][tile tt][c]
    z0r = np.ascontiguousarray(
        z0p.reshape(KCH, TPC, 128, C).transpose(0, 2, 1, 3)
    ).reshape(KCH, 128, TPC * C)
    # block-diagonal W2 tile: diag(W2, W2) handles two batches at once
    w_sb = np.zeros((128, 128), dtype=BF16)
    w_sb[0:F, 0:F] = w[1].astype(BF16)
    w_sb[F:2 * F, F:2 * F] = w[1].astype(BF16)
    in_maps = []
    for j in range(NCORES):
        at_j = a_bf[j * M_CORE:(j + 1) * M_CORE, :].T   # [N, M_CORE]
        # repack: [mh][chunk j][partition p][tile tt][m]
        atr = np.ascontiguousarray(
            at_j.reshape(KCH, TPC, 128, 2, 512).transpose(3, 0, 2, 1, 4)
        ).reshape(2, KCH, 128, TPC * 512)
        # rotate contraction chunks so peer (j+1+s)%8 sits at slot s and
        # the core's own n-rows at slot 7 (must match the z0 rotation)
        rot = [(j + 1 + s) % NCORES for s in range(NSLOT)] + [j]
        atr = np.ascontiguousarray(atr[:, rot]).reshape(
            2 * KCH, 128, TPC * 512)
        in_maps.append({"atr": atr, "z0r": np.ascontiguousarray(z0r[rot]),
                        "w": w_sb})
    return in_maps


def kernel(x, t, net_params, A):
    x = np.asarray(x)
    A = np.asarray(A)
    net_params = np.asarray(net_params)

    if "nc" not in _CACHED:
        _CACHED["nc"] = _build_program()
    nc = _CACHED["nc"]

    in_maps = _prep_inputs(x, net_params, A)
    _CACHED["in_maps"] = in_maps
    res = run_bass_kernel_spmd(nc, in_maps, list(range(NCORES)))
    # out per core: [128, (mh, ch) * 512] = z2^T -> [M_CORE, C]
    parts = []
    for c in range(NCORES):
        o = np.asarray(res.results[c]["out"]).reshape(128, 2, 2, 512)
        parts.append(np.ascontiguousarray(
            o.transpose(1, 3, 2, 0)).reshape(M_CORE, C))
    full = np.concatenate(parts, axis=0).astype(np.float32)
    return np.ascontiguousarray(full.reshape(N, B, F).transpose(1, 0, 2))


# revision 15
# speedup vs baseline: 1.0469x; 1.0469x over previous
"""Trainium2 Bass kernel for a 2-layer dense GCN block:

    z = x.reshape(B, N, F)                     # B=4, N=8192, F=64
    for i in range(2):
        z = relu((A @ z) @ W_i)                # A: [N, N] dense
    return z

Strategy (8 NeuronCores, SPMD):
  * Shard the output rows (m) of A @ Z across cores: core j owns rows
    [1024*j, 1024*(j+1)) and keeps its A^T column-slice (bf16, 16 MiB)
    resident in SBUF for BOTH layers, so A is read from HBM exactly once.
  * HOST-SIDE REPACK: A^T / Z0 / the output use per-partition contiguous
    tiled layouts (8 KiB DMA lines).  W1 is FOLDED INTO Z0 on the host
    ((A z) W1 = A (z W1)); layer 1's tail applies relu then the
    block-diag W2 ((A z1) W2 = A (z1 W2)), and layer 2's tail is a bare
    relu with the output left in [c, m] layout the host untangles.
  * ROTATED PER-CORE CHUNK LAYOUT: core c's A^T / Z0 contraction chunks
    are host-reordered to [(c+1+j) % 8 for j in 0..6] + [c], putting the
    core's OWN n-rows at static slot 7.  Layer 1 is order-invariant; for
    layer 2 this makes the 8 "self" n-tiles addressable from the LOCAL
    z1' send tiles with no gather dependency.  They are issued first, so
    the PE crunches ~9us of real work in the window where it previously
    idled waiting for the first gather (whose ~18us stall re-throttled
    the HAM clock gate to 1.2 GHz for the start of layer 2).
  * The gathered peer blocks are restaged with DYNAMIC row offsets
    (partition_id + ds): slot s <- gathered block (rank+1+s) % 8, so the
    slot order matches the rotated A^T layout on every core.
  * Z is [n, c] with c = b*F + f (256 columns).  Layer matmuls compute
    H^T[c, m] = sum_n Z[n, c] * A^T[n, m] (lhsT = Z tile, rhs = A^T
    half tile [128, 512], fp32 PSUM accum).
  * Layer 1 runs as TWO m-half passes in DMA arrival order, one 1 MiB
    AllGather per half (CC ops have a ~13us serial floor; NG=4 measured
    worse).  mh=1 stores ride the gpsimd queue (its DMA-completion
    semaphore pool is untouched by load/restage traffic).
  * A 1-element relu at kernel start absorbs the ~2.7us ACT_TABLE_LOAD.
  * bf16 operands / fp32 accumulation (measured ~0.5% rel-l2 vs the
    fp32 reference).  Final output is fp32 (reassembled on the host).
"""

import contextlib

import numpy as np
import ml_dtypes

import concourse.mybir as mybir
import concourse.tile as tile
from concourse import bacc
from concourse.bass import ds
from concourse.bass_utils import run_bass_kernel_spmd

BF16 = ml_dtypes.bfloat16

NCORES = 8
B, N, F, L = 4, 8192, 64, 2
C = B * F                      # 256 columns of the Z matrix
M_CORE = N // NCORES           # 1024 output rows per core
NT = N // 128                  # 64 contraction tiles of 128
MT = M_CORE // 128             # 8 output-row tiles of 128 per core
NG = 2                         # one AllGather per m-half
MPG = MT // NG                 # m-tiles per gather slice (4)
TPC = 8                        # n-tiles per DMA chunk
KCH = NT // TPC                # 8 chunks
NSLOT = NCORES - 1             # restaged peer slots (self excluded)
STAG2 = 16                     # layer-2 tail sweep tiles

_CACHED = {}


def _build_program():
    nc = bacc.Bacc("TRN2", target_bir_lowering=False, debug=False,
                   num_devices=NCORES)
    dt = mybir.dt

    # host-repacked inputs: per-partition-contiguous tiled layouts
    atr_d = nc.dram_tensor("atr", [2 * KCH, 128, TPC * 512], dt.bfloat16,
                           kind="ExternalInput")
    z0r_d = nc.dram_tensor("z0r", [KCH, 128, TPC * C], dt.bfloat16,
                           kind="ExternalInput")
    w_d = nc.dram_tensor("w", [128, 128], dt.bfloat16, kind="ExternalInput")
    # output: z2^T slabs [c-in-ch, (mh, ch) x 512 m]; host reassembles
    out_d = nc.dram_tensor("out", [128, 4 * 512], dt.bfloat16,
                           kind="ExternalOutput")

    z1_loc = [nc.dram_tensor(f"z1_loc{g}", [128, MPG * C], dt.bfloat16)
              for g in range(NG)]
    z1g = [nc.dram_tensor(f"z1g{g}", [NCORES * 128, MPG * C], dt.bfloat16,
                          addr_space="Shared")
           for g in range(NG)]

    with tile.TileContext(nc) as tc:
        with tc.tile_pool(name="a_res", bufs=1) as a_pool, \
             tc.tile_pool(name="z_res", bufs=1) as z_pool, \
             tc.tile_pool(name="z1_res", bufs=1) as z1_pool, \
             tc.tile_pool(name="snd", bufs=1) as snd_pool, \
             tc.tile_pool(name="wk", bufs=1) as w_pool, \
             tc.tile_pool(name="ht", bufs=3, space="PSUM") as psh_pool, \
             tc.tile_pool(name="pz", bufs=2, space="PSUM") as psz_pool, \
             tc.tile_pool(name="hsb", bufs=2) as hsb_pool:

            # per-engine core rank + rotated gather-block indices for the
            # dynamic restages: slot s holds peer (rank+1+s) % 8
            p_sync = nc.sync.partition_id()
            p_gps = nc.gpsimd.partition_id()
            bs_sync = [nc.sync.snap((p_sync + 1 + s) % NCORES,
                                    min_val=0, max_val=NCORES - 1)
                       for s in range(NSLOT)]
            b0_gps = nc.gpsimd.snap((p_gps + 1) % NCORES,
                                    min_val=0, max_val=NCORES - 1)

            w_sb = w_pool.tile([128, 128], dt.bfloat16, tag="w")
            nc.scalar.dma_start(out=w_sb[:], in_=w_d[:])
            # ACT_TABLE_LOAD warmup: first relu pays ~2.7us table load
            wrm_sb = w_pool.tile([128, 1], dt.bfloat16, tag="wrm")
            nc.scalar.activation(wrm_sb[:], w_sb[:, :1],
                                 mybir.ActivationFunctionType.Relu)

            ath_sb = {(mh, j): a_pool.tile([128, TPC * 512], dt.bfloat16,
                                           tag=f"ath{mh}{j}",
                                           name=f"ath_sb{mh}{j}")
                      for mh in range(2) for j in range(KCH)}
            z_sb = [z_pool.tile([128, TPC * C], dt.bfloat16,
                                tag=f"z{j}", name=f"z_sb{j}")
                    for j in range(KCH)]
            z1_sb = [z1_pool.tile([128, NSLOT * MPG * C], dt.bfloat16,
                                  tag=f"z1s{g}", name=f"z1_sb{g}")
                     for g in range(NG)]
            # contiguous per-half send tiles (tails write slices of these;
            # layer 2's self n-tiles also read them directly)
            z1snd = [snd_pool.tile([128, MPG * C], dt.bfloat16,
                                   tag=f"z1snd{mh}", name=f"z1snd{mh}")
                     for mh in range(2)]
            z2snd = snd_pool.tile([128, 4 * 512], dt.bfloat16,
                                  tag="z2snd", name="z2snd")

            # pass-0 feed: Z0 rides the scalar queue (done by ~25us, well
            # before the tail's store DMAs need it) while A-mh0 rides
            # sync -- two DGE queues push descriptors through the slow
            # first-5us ramp instead of one.  First chunk split so the
            # first matmul starts ~6us earlier.
            nc.scalar.dma_start(out=z_sb[0][:, :C], in_=z0r_d[0][:, :C])
            nc.sync.dma_start(out=ath_sb[0, 0][:, :512],
                              in_=atr_d[0][:, :512])
            nc.scalar.dma_start(out=z_sb[0][:, C:2 * C],
                                in_=z0r_d[0][:, C:2 * C])
            nc.sync.dma_start(out=ath_sb[0, 0][:, 512:2 * 512],
                              in_=atr_d[0][:, 512:2 * 512])
            nc.scalar.dma_start(out=z_sb[0][:, 2 * C:],
                                in_=z0r_d[0][:, 2 * C:])
            nc.sync.dma_start(out=ath_sb[0, 0][:, 2 * 512:],
                              in_=atr_d[0][:, 2 * 512:])
            for j in range(1, KCH):
                nc.scalar.dma_start(out=z_sb[j][:], in_=z0r_d[j])
                nc.sync.dma_start(out=ath_sb[0, j][:], in_=atr_d[j])

            def z0_tile(t, ch):
                """lhsT: Z0[n-slot-tile t, c-half ch] -> [128, 128] bf16."""
                j, tt = divmod(t, TPC)
                return z_sb[j][:, tt * C + ch * 128: tt * C + ch * 128 + 128]

            def z1_tile(t, ch):
                """lhsT: restaged peer Z1 for slot-tile t (slot 0..6)."""
                s, r = divmod(t, MT)
                g, i = divmod(r, MPG)
                base = s * (MPG * C) + i * C + ch * 128
                return z1_sb[g][:, base: base + 128]

            def z1self_tile(r, ch):
                """lhsT: the core's own Z1 (slot 7) straight from the
                send tiles -- no gather dependency."""
                mh, io = divmod(r, MPG)
                return z1snd[mh][:, io * C + ch * 128: io * C + ch * 128 + 128]

            def at_tile(t, mh):
                """rhs: A^T[n-slot-tile t, m-half mh] -> [128, 512] bf16."""
                j, tt = divmod(t, TPC)
                return ath_sb[mh, j][:, tt * 512:(tt + 1) * 512]

            def l1_tail(mh, h_ps, hook):
                """relu(h1) -> bf16, apply block-diag W2 (which also
                transposes [c,m] -> [m,c]), round into the half's send
                tile, then the caller's per-m-tile hook."""
                with tc.high_priority():
                    hr = [hsb_pool.tile([128, 512], dt.bfloat16,
                                        tag=f"h{ch}", name=f"hr_sb_{ch}{mh}")
                          for ch in range(2)]
                    for ch in range(2):
                        nc.scalar.activation(
                            hr[ch][:], h_ps[ch, mh][:],
                            mybir.ActivationFunctionType.Relu)
                    for io in range(MPG):
                        sl = slice(io * 128, (io + 1) * 128)
                        z_ps = psz_pool.tile([128, C], dt.float32, tag="zps",
                                             name=f"z_ps_{mh}{io}")
                        for ch in range(2):
                            nc.tensor.matmul(
                                z_ps[:, ch * 128:(ch + 1) * 128],
                                hr[ch][:, sl], w_sb[:],
                                start=True, stop=True)
                        nc.vector.tensor_copy(
                            z1snd[mh][:, io * C:(io + 1) * C], z_ps[:])
                        hook(mh, io, io + 1)

            # ---- layer 1: two m-half passes in DMA arrival order ----
            def l1_hook(mh, lo, hi):
                # 2KiB-line store of part of the half's z1'; after the
                # last part, the gather and the rotated per-slot restage
                # into SBUF.  mh=1's stores go on the gpsimd queue: its
                # DMA-completion semaphore pool is untouched by the
                # load/restage traffic, so the stores cannot inherit a
                # ring-wait on a gather-gated DMA.
                st_eng = nc.sync if mh == 0 else nc.gpsimd
                st_eng.dma_start(out=z1_loc[mh][:, lo * C:hi * C],
                                 in_=z1snd[mh][:, lo * C:hi * C])
                if hi < MPG:
                    return
                nc.gpsimd.collective_compute(
                    "AllGather",
                    mybir.AluOpType.bypass,
                    replica_groups=[list(range(NCORES))],
                    ins=[z1_loc[mh].ap().opt()],
                    outs=[z1g[mh].ap().opt()],
                )
                # Slot 0 (peer rank+1) in two halves so layer 2's first
                # post-self matmul sees minimum staging latency.  g0's
                # pair is deferred to after gather-1's doorbell, behind a
                # scheduler fence, and rides the gpsimd queue.
                if mh == 1:
                    tc.no_sync_barrier()
                    for g, eng, bsv in ((0, nc.gpsimd, b0_gps),
                                        (1, nc.sync, bs_sync[0])):
                        eng.dma_start(
                            out=z1_sb[g][:, :MPG * C // 2],
                            in_=z1g[g][ds(bsv * 128, 128)][:,
                                                           :MPG * C // 2])
                        eng.dma_start(
                            out=z1_sb[g][:, MPG * C // 2:MPG * C],
                            in_=z1g[g][ds(bsv * 128, 128)][:,
                                                           MPG * C // 2:])
                for s in range(1, NSLOT):
                    nc.sync.dma_start(
                        out=z1_sb[mh][:, s * MPG * C:(s + 1) * MPG * C],
                        in_=z1g[mh][ds(bs_sync[s] * 128, 128)])

            h_ps1 = {(ch, mh): psh_pool.tile([128, 512], dt.float32,
                                             tag=f"hps{ch}",
                                             name=f"hps_1_{ch}{mh}")
                     for ch in range(2) for mh in range(2)}
            for mh in range(2):
                for t in range(NT):
                    for ch in range(2):
                        nc.tensor.matmul(
                            h_ps1[ch, mh][:], z0_tile(t, ch), at_tile(t, mh),
                            start=(t == 0), stop=(t == NT - 1))
                if mh == 0:
                    # first two A-mh1 chunks ahead of the mh0 z1 stores
                    # (pass 1 needs them before the stores' data exists)
                    for j in range(2):
                        nc.sync.dma_start(out=ath_sb[1, j][:],
                                          in_=atr_d[KCH + j])
                l1_tail(mh, h_ps1, l1_hook)
                if mh == 0:
                    # rest of the A-mh1 stream AFTER the mh0 z1 stores so
                    # the stores' packets do not trail the 8MiB load
                    for j in range(2, KCH):
                        nc.sync.dma_start(out=ath_sb[1, j][:],
                                          in_=atr_d[KCH + j])

            # ---- layer 2: self-first, then gather arrival order ----
            h_ps2 = {(ch, mh): psh_pool.tile([128, 512], dt.float32,
                                             tag=f"hps{ch}",
                                             name=f"hps_2_{ch}{mh}")
                     for ch in range(2) for mh in range(2)}
            # Scheduler fence: orders every engine's queue here so layer
            # 2's matmuls cannot head-of-line block layer-1 mh1's tail /
            # the second gather's doorbell.
            tc.no_sync_barrier()
            # self n-tiles (slot 7): local z1' only -- these fill the PE
            # gap while the first gather is still in flight, keeping the
            # HAM clock gate warm
            for r in range(MT):
                for ch in range(2):
                    for mh in range(2):
                        nc.tensor.matmul(
                            h_ps2[ch, mh][:], z1self_tile(r, ch),
                            at_tile(NSLOT * TPC + r, mh),
                            start=(r == 0), stop=False)
            t2 = [s * MT + g * MPG + i
                  for g in range(NG) for s in range(NSLOT) for i in range(MPG)]
            head, sweep = t2[:len(t2) - STAG2], t2[len(t2) - STAG2:]
            for t in head:
                for ch in range(2):
                    for mh in range(2):
                        nc.tensor.matmul(
                            h_ps2[ch, mh][:], z1_tile(t, ch), at_tile(t, mh),
                            start=False, stop=False)
            for mh in range(2):
                for si, t in enumerate(sweep):
                    for ch in range(2):
                        nc.tensor.matmul(
                            h_ps2[ch, mh][:], z1_tile(t, ch), at_tile(t, mh),
                            start=False, stop=(si == STAG2 - 1))
                prio = (contextlib.nullcontext() if mh == 1
                        else tc.high_priority())
                with prio:
                    for ch in range(2):
                        off = (mh * 2 + ch) * 512
                        nc.scalar.activation(
                            z2snd[:, off:off + 512],
                            h_ps2[ch, mh][:],
                            mybir.ActivationFunctionType.Relu)
                        nc.scalar.dma_start(
                            out=out_d[:, off:off + 512],
                            in_=z2snd[:, off:off + 512])

    nc.compile()
    return nc


def _prep_inputs(x, net_params, A):
    a_bf = A.astype(BF16)
    w = net_params.astype(np.float32).reshape(L, F, F)
    z0 = np.ascontiguousarray(x.transpose(1, 0, 2))          # [N, B, F] f32
    # fold W1 into z0:  (A z0) W1 == A (z0 W1)
    z0p = np.einsum('nbf,fg->nbg', z0, w[0]).reshape(N, C).astype(BF16)
    # z0 repack: [chunk j][partition p][tile tt][c]
    z0r = np.ascontiguousarray(
        z0p.reshape(KCH, TPC, 128, C).transpose(0, 2, 1, 3)
    ).reshape(KCH, 128, TPC * C)
    # block-diagonal W2 tile: diag(W2, W2) handles two batches at once
    w_sb = np.zeros((128, 128), dtype=BF16)
    w_sb[0:F, 0:F] = w[1].astype(BF16)
    w_sb[F:2 * F, F:2 * F] = w[1].astype(BF16)
    in_maps = []
    for j in range(NCORES):
        at_j = a_bf[j * M_CORE:(j + 1) * M_CORE, :].T   # [N, M_CORE]
        # repack: [mh][chunk j][partition p][tile tt][m]
        atr = np.ascontiguousarray(
            at_j.reshape(KCH, TPC, 128, 2, 512).transpose(3, 0, 2, 1, 4)
        ).reshape(2, KCH, 128, TPC * 512)
        # rotate contraction chunks so peer (j+1+s)%8 sits at slot s and
        # the core's own n-rows at slot 7 (must match the z0 rotation)
        rot = [(j + 1 + s) % NCORES for s in range(NSLOT)] + [j]
        atr = np.ascontiguousarray(atr[:, rot]).reshape(
            2 * KCH, 128, TPC * 512)
        in_maps.append({"atr": atr, "z0r": np.ascontiguousarray(z0r[rot]),
                        "w": w_sb})
    return in_maps


def kernel(x, t, net_params, A):
    x = np.asarray(x)
    A = np.asarray(A)
    net_params = np.asarray(net_params)

    if "nc" not in _CACHED:
        _CACHED["nc"] = _build_program()
    nc = _CACHED["nc"]

    in_maps = _prep_inputs(x, net_params, A)
    _CACHED["in_maps"] = in_maps
    res = run_bass_kernel_spmd(nc, in_maps, list(range(NCORES)))
    # out per core: [128, (mh, ch) * 512] = z2^T -> [M_CORE, C]
    parts = []
    for c in range(NCORES):
        o = np.asarray(res.results[c]["out"]).reshape(128, 2, 2, 512)
        parts.append(np.ascontiguousarray(
            o.transpose(1, 3, 2, 0)).reshape(M_CORE, C))
    full = np.concatenate(parts, axis=0).astype(np.float32)
    return np.ascontiguousarray(full.reshape(N, B, F).transpose(1, 0, 2))
